# revision 3
# baseline (speedup 1.0000x reference)
"""Trainium2 Bass kernel for an 8-head cross-attention block.

Math (per reference):
    Q = video @ Wq[h]           [4096, 64]  per head
    K = text  @ Wk[h]           [1024, 64]
    V = text  @ Wv[h]           [1024, 64]
    att = softmax(Q @ K^T)      [4096, 1024]   (no scaling)
    y_h = att @ V               [4096, 64]
    out = concat_h(y_h) @ Wout + pos_enc(4096, 512)

Sharding: head-parallel over 8 NeuronCores. Core h owns head h and the
matching 64 rows of Wout (row-parallel); the all-reduce over cores and the
positional-encoding add happen on host during the gather.

v3 layout (rebuilt from the v2 trace):
  * Weights ship host-packed in ONE contiguous [128, 768] fp16 buffer
    (v2's per-weight rearranged DMAs moved 128B packets and took 5-8us).
  * ~24 junk matmuls at the top keep the PE busy from preamble end so the
    HAM clock gate reaches 8/8 before real work, and soak the tt DMA wait.
  * Softmax den: row 64 of att@V (ones column in V'), then per j-chunk:
    DVE reciprocal of the [1,512] den row, K=1 ones-matmul broadcast to
    [64,512] PSUM, one DVE in-place multiply on y65. Replaces v2's DRAM
    round-trip transpose + per-token scaling of every out-proj tile.
  * out_proj lags Y by ONE iteration (v2: two); output DMAs are per-512-row
    chunk so the final DMA tail is ~1.4us.
  * All PSUM drains are DVE/ACT (GpSimd has no PSUM port); vt DMA is
    chunked so Q-proj j0 starts as soon as its tokens land.
"""

import numpy as np

from concourse import bacc
import concourse.mybir as mybir
from concourse.tile import TileContext
from concourse.bass_utils import run_bass_kernel_spmd

N, M, D, H, DH = 4096, 1024, 512, 8, 64
P = 128
NC = 512          # n-chunk width (one j)
NJ = N // NC      # 8 n-chunks
NT = NJ // 2      # 4 j-pairs
DC = D // P       # 4 contraction chunks of 128
MT = M // P       # 8 key tiles of 128
F32 = mybir.dt.float32
FP16 = mybir.dt.float16
EXP = mybir.ActivationFunctionType.Exp
EXP_SHIFT = -12.0  # exp(E + shift): keeps exp in fp16 range; cancels in softmax
NCORES = 8
NWARM = 24         # junk matmuls to hold PE busy through the HAM window

_CACHE: dict = {}
TRACE = False          # test harness can flip this before calling kernel()
LAST_RESULT = None     # BassKernelResults of the last run (for profiling)


def _body(tc, nc, vT, tT, wcat, wo, out):
    with tc.tile_pool(name="const", bufs=1) as cp, \
         tc.tile_pool(name="pbuf", bufs=16) as pp, \
         tc.tile_pool(name="obuf", bufs=3) as op, \
         tc.tile_pool(name="ps_e", bufs=2, space="PSUM") as pe_pool, \
         tc.tile_pool(name="ps_y", bufs=2, space="PSUM") as py_pool, \
         tc.tile_pool(name="ps_m", bufs=2, space="PSUM") as pm_pool:

        tt_sb = cp.tile([P, DC * M], FP16, tag="tt")
        vt_sb = cp.tile([P, DC * N], FP16, tag="vt")
        wcat_sb = cp.tile([P, 3 * DC * DH], FP16, tag="wcat")
        wo_sb = cp.tile([DH, D], FP16, tag="wo")
        kt_sb = cp.tile([P, M], FP16, tag="kt")        # K^T duplicated on halves
        qt_sb = cp.tile([P, NT * NC], FP16, tag="qt")  # rows 0:64 j-even, 64:128 j-odd
        v3 = cp.tile([P, MT * (DH + 1)], FP16, tag="v3")
        y65 = cp.tile([DH + 1, NJ * NC], FP16, tag="y65")  # row 64 = fp16 den
        bias_sb = cp.tile([P, 1], F32, tag="bias")
        ones_sb = cp.tile([1, DH], FP16, tag="ones")
        rcrow = cp.tile([1, 2 * NC], FP16, tag="rcrow")  # recip den, per parity
        junk = cp.tile([P, 256], FP16, tag="junk")

        tt3 = tt_sb.rearrange("p (c m) -> p c m", c=DC)
        vt3 = vt_sb.rearrange("p (c n) -> p c n", c=DC)
        wq3 = wcat_sb.rearrange("p (w c e) -> p w c e", w=3, c=DC)[:, 0]
        wk3 = wcat_sb.rearrange("p (w c e) -> p w c e", w=3, c=DC)[:, 1]
        wv3 = wcat_sb.rearrange("p (w c e) -> p w c e", w=3, c=DC)[:, 2]
        v3r = v3.rearrange("p (m e) -> p m e", e=DH + 1)

        # ---- constants + input DMAs ----
        nc.vector.memset(junk[:, :], 0.0)
        nc.vector.memset(bias_sb[:, :], EXP_SHIFT)
        nc.vector.memset(ones_sb[:, :], 1.0)
        nc.vector.memset(v3r[:, :, DH], 1.0)

        nc.scalar.dma_start(out=wcat_sb[:, :], in_=wcat[:, :])
        nc.scalar.dma_start(out=wo_sb[:, :], in_=wo[:, :])
        nc.sync.dma_start(out=tt3[:, :, :], in_=tT.rearrange("(c p) m -> p c m", p=P))
        vTr = vT.rearrange("(c p) n -> p c n", p=P)
        # first two chunks are single j's so Q-proj(0) can start early
        vchunks = [(0, NC), (NC, 2 * NC), (2 * NC, 4 * NC),
                   (4 * NC, 6 * NC), (6 * NC, 8 * NC)]
        for lo, hi in vchunks:
            nc.gpsimd.dma_start(out=vt3[:, :, lo:hi], in_=vTr[:, :, lo:hi])

        # ---- PE warmup: junk matmuls keep HAM busy while tt streams ----
        jp = pm_pool.tile([P, NC], F32, tag="mm")
        for _ in range(NWARM):
            nc.tensor.matmul(jp[:, 0:256], junk[:, 0:128], junk[:, :],
                             start=True, stop=True)

        # ---- K proj: col-tiled pair (m-halves in PE col groups) ----
        psk = pm_pool.tile([P, NC], F32, tag="mm")
        for c in range(DC):
            nc.tensor.matmul(psk[0:DH, :], wk3[:, c, :], tt3[:, c, 0:512],
                             start=(c == 0), stop=(c == DC - 1))
            nc.tensor.matmul(psk[DH:P, :], wk3[:, c, :], tt3[:, c, 512:1024],
                             start=(c == 0), stop=(c == DC - 1))
        nc.vector.tensor_copy(out=kt_sb[0:DH, 0:512], in_=psk[0:DH, :])
        nc.vector.tensor_copy(out=kt_sb[DH:P, 0:512], in_=psk[0:DH, :])
        nc.vector.tensor_copy(out=kt_sb[0:DH, 512:1024], in_=psk[DH:P, :])
        nc.vector.tensor_copy(out=kt_sb[DH:P, 512:1024], in_=psk[DH:P, :])

        # ---- V proj: all 8 m-tiles into one PSUM bank ----
        psv = pm_pool.tile([P, NC], F32, tag="mm")
        for mt in range(MT):
            for c in range(DC):
                nc.tensor.matmul(psv[:, mt * DH:(mt + 1) * DH],
                                 tt3[:, c, mt * P:(mt + 1) * P], wv3[:, c, :],
                                 start=(c == 0), stop=(c == DC - 1))
        nc.vector.tensor_copy(out=v3r[:, :, 0:DH],
                              in_=psv.rearrange("p (m e) -> p m e", e=DH))

        # ---- Q proj per j-pair: col-tiled pair (j-even/j-odd) ----
        def q_proj(t):
            psq = pm_pool.tile([P, NC], F32, tag="mm")
            for c in range(DC):
                nc.tensor.matmul(psq[0:DH, :], wq3[:, c, :],
                                 vt3[:, c, (2 * t) * NC:(2 * t + 1) * NC],
                                 start=(c == 0), stop=(c == DC - 1))
                nc.tensor.matmul(psq[DH:P, :], wq3[:, c, :],
                                 vt3[:, c, (2 * t + 1) * NC:(2 * t + 2) * NC],
                                 start=(c == 0), stop=(c == DC - 1))
            nc.vector.tensor_copy(out=qt_sb[:, t * NC:(t + 1) * NC], in_=psq[:, :])

        q_proj(0)

        # ---- attention + output projection, software-pipelined over t ----
        def emit_y_mms(p_list, y_ps, mts):
            for mt in mts:
                pt = p_list[mt // 2]
                nc.tensor.matmul(y_ps[:, :], v3r[:, mt, :],
                                 pt[:, (mt % 2) * NC:(mt % 2 + 1) * NC],
                                 start=(mt == 0), stop=(mt == MT - 1))

        out_r = out.rearrange("(g p) d -> p g d", p=P)  # [128, 32, 512]

        def finish(t, y_ev, y_od):
            # drain Y(t), normalize by the den row, project, stage, DMA out
            for parity, y_ps in ((0, y_ev), (1, y_od)):
                j = 2 * t + parity
                jsl = slice(j * NC, (j + 1) * NC)
                psl = slice(parity * NC, (parity + 1) * NC)
                nc.vector.tensor_copy(out=y65[:, jsl], in_=y_ps[:, :])
                with nc.allow_low_precision(reason="fp16 softmax recip, 2e-2 budget"):
                    nc.vector.reciprocal(rcrow[:, psl], y65[DH:DH + 1, jsl])
            for parity in (0, 1):
                j = 2 * t + parity
                jsl = slice(j * NC, (j + 1) * NC)
                psl = slice(parity * NC, (parity + 1) * NC)
                rcps = pm_pool.tile([P, NC], F32, tag="mm")
                nc.tensor.matmul(rcps[0:DH, :], ones_sb[:, :], rcrow[:, psl],
                                 start=True, stop=True)
                nc.vector.tensor_mul(y65[0:DH, jsl], y65[0:DH, jsl],
                                     rcps[0:DH, :])
                ot = op.tile([P, 4 * D], FP16, tag="ot")
                for g in range(4):
                    po = pm_pool.tile([P, D], F32, tag="mm")
                    nc.tensor.matmul(
                        po[:, :],
                        y65[0:DH, j * NC + g * P: j * NC + (g + 1) * P],
                        wo_sb[:, :], start=True, stop=True)
                    osl = slice(g * D, (g + 1) * D)
                    if g % 2 == 0:
                        nc.scalar.copy(ot[:, osl], po[:, :])
                    else:
                        nc.vector.tensor_copy(out=ot[:, osl], in_=po[:, :])
                nc.sync.dma_start(
                    out=out_r[:, 4 * j:4 * (j + 1), :],
                    in_=ot.rearrange("p (g d) -> p g d", d=D))

        prev_p = None   # p tiles of the previous j-pair: [(ev, od) x 4]
        prev_y = None
        for t in range(NT):
            tsl = slice(t * NC, (t + 1) * NC)
            if prev_p is not None:
                y_ev = py_pool.tile([DH + 1, NC], F32, tag="y")
                y_od = py_pool.tile([DH + 1, NC], F32, tag="y")
            cur_p = []
            for i in range(4):
                e_ev = pe_pool.tile([P, 2 * NC], F32, tag="e")
                e_od = pe_pool.tile([P, 2 * NC], F32, tag="e")
                for k in range(2):
                    mt = 2 * i + k
                    msl = slice(mt * P, (mt + 1) * P)
                    nc.tensor.matmul(e_ev[:, k * NC:(k + 1) * NC],
                                     kt_sb[0:DH, msl], qt_sb[0:DH, tsl],
                                     start=True, stop=True)
                    nc.tensor.matmul(e_od[:, k * NC:(k + 1) * NC],
                                     kt_sb[DH:P, msl], qt_sb[DH:P, tsl],
                                     start=True, stop=True)
                p_ev = pp.tile([P, 2 * NC], FP16, tag="p")
                p_od = pp.tile([P, 2 * NC], FP16, tag="p")
                nc.scalar.activation(p_ev[:, :], e_ev[:, :], EXP, bias=bias_sb[:, :])
                nc.scalar.activation(p_od[:, :], e_od[:, :], EXP, bias=bias_sb[:, :])
                cur_p.append((p_ev, p_od))
                if prev_p is not None:
                    mts = (2 * i, 2 * i + 1)
                    emit_y_mms([a for a, b in prev_p], y_ev, mts)
                    emit_y_mms([b for a, b in prev_p], y_od, mts)
            if t + 1 < NT:
                q_proj(t + 1)
            if prev_p is not None:
                finish(t - 1, y_ev, y_od)
            prev_p = cur_p
        # tail: att@V + finish for the last j-pair
        y_ev = py_pool.tile([DH + 1, NC], F32, tag="y")
        y_od = py_pool.tile([DH + 1, NC], F32, tag="y")
        for i in range(4):
            mts = (2 * i, 2 * i + 1)
            emit_y_mms([a for a, b in prev_p], y_ev, mts)
            emit_y_mms([b for a, b in prev_p], y_od, mts)
        finish(NT - 1, y_ev, y_od)


def _build():
    nc = bacc.Bacc("TRN2", target_bir_lowering=False, debug=False)
    vT = nc.dram_tensor("vT", [D, N], FP16, kind="ExternalInput")
    tT = nc.dram_tensor("tT", [D, M], FP16, kind="ExternalInput")
    wcat = nc.dram_tensor("wcat", [P, 3 * DC * DH], FP16, kind="ExternalInput")
    wo = nc.dram_tensor("wo", [DH, D], FP16, kind="ExternalInput")
    out = nc.dram_tensor("out", [N, D], FP16, kind="ExternalOutput")
    with TileContext(nc) as tc:
        _body(tc, nc, vT[:, :], tT[:, :], wcat[:, :], wo[:, :], out[:, :])
    nc.compile()
    return nc


def _pos_encoding():
    # Mirror the reference's jnp ops bit-for-bit (numpy's f32 sin/exp differ
    # by enough ULPs to dominate the error budget at pos/freq ~ 4e3).
    import jax
    import jax.numpy as jnp
    with jax.default_device(jax.devices("cpu")[0]):
        pos = jnp.arange(N, dtype=jnp.float32)
        freq = jnp.exp(
            (jnp.arange(D // 2, dtype=jnp.float32) / D)
            * jnp.log(jnp.float32(10000.0)))
        x = pos[:, None] / freq
        pe = jnp.stack((jnp.sin(x), jnp.cos(x)), axis=-1)
        return np.asarray(pe.reshape(N, D), dtype=np.float32)


def _fp16(a):
    return np.ascontiguousarray(np.asarray(a, dtype=np.float32).astype(np.float16))


def _pack_weights(Wq, Wk, Wv, h):
    # [128, 768]: for w in (q,k,v), cols w*256 + c*64 + e = W[h][c*128+p, e]
    cols = []
    for W in (Wq, Wk, Wv):
        w = np.asarray(W[h], dtype=np.float32)          # [512, 64]
        cols.append(w.reshape(DC, P, DH).transpose(1, 0, 2).reshape(P, DC * DH))
    return _fp16(np.concatenate(cols, axis=1))


def kernel(video_features, text_features, Wq, Wk, Wv, Wout):
    global LAST_RESULT
    if "nc" not in _CACHE:
        _CACHE["nc"] = _build()
        _CACHE["pe"] = _pos_encoding()
    nc = _CACHE["nc"]

    vT = _fp16(np.asarray(video_features, dtype=np.float32).T)
    tT = _fp16(np.asarray(text_features, dtype=np.float32).T)
    Wout = np.asarray(Wout, dtype=np.float32)

    in_maps = []
    for h in range(NCORES):
        in_maps.append({
            "vT": vT,
            "tT": tT,
            "wcat": _pack_weights(Wq, Wk, Wv, h),
            "wo": _fp16(Wout[h * DH:(h + 1) * DH, :]),
        })
    res = run_bass_kernel_spmd(nc, in_maps, list(range(NCORES)), trace=TRACE)
    LAST_RESULT = res
    acc = res.results[0]["out"].astype(np.float32)
    for h in range(1, NCORES):
        acc = acc + res.results[h]["out"].astype(np.float32)
    return (acc + _CACHE["pe"]).astype(np.float32)
